# revision 3
# baseline (speedup 1.0000x reference)
"""Trainium2 Bass kernel for ClassicalGCN message passing.

Reference computation:
    h   = tanh(x @ W1 + b1)                       # [N, HID]
    agg = segment_sum(edge_val * h[edge_col], edge_row, N)
    out = agg @ W2 + b2                           # [N, 1]

Key algebraic rewrite: W2 commutes through the linear aggregation:

    s      = tanh(x @ W1 + b1) @ W2               # [N] per-node scalar
    out[i] = b2 + sum_{e: row[e]==i} val[e] * s[col[e]]

Sharding: nodes (output rows) are split across the 8 NeuronCores; edges are
partitioned by destination row. x and the small weights are replicated; each
core computes the full s vector locally (no collectives), keeps it in SBUF
as a bf16 pair-table replicated across all 128 partitions, and gathers
per-edge values with the GPSIMD ap_gather ucode (SBUF-local gather; no
per-edge HBM traffic and no SWDGE descriptor generation).

Per-core device program:
  Phase A: s = tanh(x@W1+b1)@W2 for all nodes via 3 PE matmuls per
           1024-node chunk: two x@W1 halves (z on partitions 0:64 / 64:128),
           ACT tanh (bias fused), then ONE block-diagonal W2 matmul whose
           output holds s for the first 512 nodes replicated on partitions
           0:64 and s for the next 512 nodes on partitions 64:128. Two
           half-copies move those into the SBUF table; periodic strided
           SBUF->SBUF "fill" DMAs copy each half to the opposite partition
           range, completing the 128-way replication.
  Phase B: rows are sorted by degree (host-side permutation, undone on the
           host afterwards), 49 chunks of 128 rows are packed into batches
           with a uniform ELL width W per batch. Per batch: ONE ap_gather
           fetches the bf16 pairs for all chunks in the batch (stream laid
           out row-major within each 16-partition group), ONE strided-
           partition SBUF->SBUF DMA restripes the (16x redundant) gathered
           stream to one row per partition, and the DVE does ONE multiply
           by the preloaded 2-lane val mask plus ONE batched reduce into
           out[128, chunks].
"""

import os

import numpy as np
import ml_dtypes

import concourse.bass as bass
import concourse.mybir as mybir
import concourse.tile as tile
from concourse import bacc
from concourse.bass_utils import run_bass_kernel_spmd
from concourse.tile_rust import add_dep_helper

# Problem sizes (hardcoded per spec nn_ClassicalGCN_77077483094916)
N = 50000
E = 1600000
IN_DIM = 128
HID = 64
NCORES = 8

RPC = N // NCORES            # rows per core = 6250
RPAD = 6272                  # rows padded to 128*49
NCHUNK = RPAD // 128         # 49 row chunks of 128
NPAD = 50176                 # nodes padded to 49*1024
NPAIR = NPAD // 2            # bf16 pairs in the s table
ACHUNKS = NPAD // 1024       # 49 phase-A iterations
FILLGRP = 7                  # phase-A chunks per cross-fill DMA pair
BATCH_CAP = 320              # max B*W (pair columns) per phase-B batch
BATCH_MAXB = 16              # max chunks per batch

F32 = mybir.dt.float32
BF16 = mybir.dt.bfloat16
I16 = mybir.dt.int16

_LAST_RESULTS = {"exec_time_ns": None}


def _install_ntff_hook():
    """Register the axon NTFF profile hook if the image's antenv lacks it.

    bass_utils reads `antenv.axon_hooks.get_axon_ntff_profile_hook` lazily
    when trace=True under axon; this image's antenv has no axon_hooks
    module, so synthesize one around direct ctypes calls into
    libaxon_pjrt.so (same ABI trn_boot._ntff_profile_via_ctypes uses).
    """
    import sys

    if "antenv.axon_hooks" in sys.modules:
        return
    import contextlib
    import ctypes
    import types

    so_path = "/opt/axon/libaxon_pjrt.so"
    if not os.path.exists(so_path):
        return
    try:
        lib = ctypes.CDLL(so_path)
    except OSError:
        return
    if not hasattr(lib, "axon_start_nrt_profile"):
        return
    lib.axon_start_nrt_profile.argtypes = [
        ctypes.POINTER(ctypes.c_int64),
        ctypes.c_size_t,
    ]
    lib.axon_start_nrt_profile.restype = ctypes.c_int64
    lib.axon_stop_nrt_profile.argtypes = [ctypes.c_char_p]
    lib.axon_stop_nrt_profile.restype = ctypes.c_int64

    @contextlib.contextmanager
    def _hook(output_dir, device_ids):
        import jax

        jax.devices()
        if device_ids:
            ids = (ctypes.c_int64 * len(device_ids))(*device_ids)
            rc = lib.axon_start_nrt_profile(ids, len(device_ids))
        else:
            rc = lib.axon_start_nrt_profile(None, 0)
        if rc != 0:
            raise RuntimeError(f"axon_start_nrt_profile rc={rc}")
        try:
            yield
        finally:
            n = lib.axon_stop_nrt_profile(str(output_dir).encode())
            if n < 0:
                raise RuntimeError(f"axon_stop_nrt_profile rc={n}")
            print(f"profile: {n} file(s) written to {output_dir}")

    mod = types.ModuleType("antenv.axon_hooks")
    mod.get_axon_ntff_profile_hook = lambda: _hook
    mod.set_axon_ntff_profile_hook = lambda h: None
    sys.modules["antenv.axon_hooks"] = mod


def _build_program(batches):
    """batches: tuple of (k0, B, W) phase-B batch descriptors."""
    sumc = sum(B * W for (_, B, W) in batches)
    nc = bacc.Bacc("TRN2", target_bir_lowering=False, debug=False)

    xT = nc.dram_tensor("xT", [128, NPAD], BF16, kind="ExternalInput")
    W1 = nc.dram_tensor("W1", [128, HID], BF16, kind="ExternalInput")
    b1c = nc.dram_tensor("b1c", [128, 1], F32, kind="ExternalInput")
    # Block-diagonal W2: out[m<64, c] = s of node c (first half), out[m>=64,
    # c] = s of node c+512 (second half), each replicated across its 64
    # output partitions.
    W2blk = nc.dram_tensor("W2blk", [128, 128], BF16, kind="ExternalInput")
    idx = nc.dram_tensor("idx", [128, sumc], I16, kind="ExternalInput")
    msk = nc.dram_tensor("msk", [128, 2 * sumc], BF16, kind="ExternalInput")
    outd = nc.dram_tensor("out", [128, NCHUNK], F32, kind="ExternalOutput")

    with tile.TileContext(nc) as tc:
        with (
            tc.tile_pool(name="const", bufs=1) as cpool,
        ):
            W1_sb = cpool.tile([128, HID], BF16)
            nc.sync.dma_start(W1_sb[:], W1[:, :])
            b1_sb = cpool.tile([128, 1], F32)
            nc.sync.dma_start(b1_sb[:], b1c[:, :])
            W2_sb = cpool.tile([128, 128], BF16)
            nc.sync.dma_start(W2_sb[:], W2blk[:, :])

            idx_sb = cpool.tile([128, sumc], I16)
            idx_ld = nc.sync.dma_start(idx_sb[:], idx[:, :])
            msk_sb = cpool.tile([128, 2 * sumc], BF16)
            nc.scalar.dma_start(msk_sb[:], msk[:, :])

            table_sb = cpool.tile([128, NPAD], BF16)
            out_sb = cpool.tile([128, NCHUNK], F32)

            # Dummy ap_gather issued first: pays the ~6us GPSIMD library
            # IRAM load during phase A instead of on the critical path.
            dtbl = cpool.tile([128, 32], BF16)
            m0 = nc.vector.memset(dtbl[:], 0)
            didx = cpool.tile([128, 1], I16)
            m1 = nc.vector.memset(didx[:], 0)
            dout = cpool.tile([128, 32], BF16)
            dg = nc.gpsimd.ap_gather(
                out_ap=dout[:].rearrange("p (n d) -> p n d", d=2),
                in_ap=dtbl[:].rearrange("p (n d) -> p n d", d=2),
                idxs_ap=didx[:, :],
                channels=128,
                num_elems=16,
                d=2,
                num_idxs=16,
            )
            add_dep_helper(dg.ins, m0.ins, reason="dummy tbl init")
            add_dep_helper(dg.ins, m1.ins, reason="dummy idx init")

            # ---- Phase A: table[:, n] = tanh(x@W1+b1)@W2, all partitions --
            table_insts = []

            def emit_fill(j0, ng):
                # cross-partition fills: replicate each 64-partition half of
                # chunks [j0, j0+ng) into the opposite partition range
                lo = table_sb[0:64, 1024 * j0 : 1024 * (j0 + ng)]
                hi = table_sb[64:128, 1024 * j0 : 1024 * (j0 + ng)]
                lov = lo.rearrange("p (g c) -> p g c", g=ng)
                hiv = hi.rearrange("p (g c) -> p g c", g=ng)
                fA = nc.sync.dma_start(hiv[:, :, 0:512], lov[:, :, 0:512])
                fB = nc.sync.dma_start(lov[:, :, 512:1024],
                                       hiv[:, :, 512:1024])
                table_insts.append(fA)
                table_insts.append(fB)

            pending_fills = []
            with (
                tc.tile_pool(name="xload", bufs=3) as xpool,
                tc.tile_pool(name="thp", bufs=2) as thpool,
                tc.tile_pool(name="pz", bufs=2, space="PSUM") as pz,
                tc.tile_pool(name="psd", bufs=2, space="PSUM") as psd,
            ):
                for i in range(ACHUNKS):
                    xt = xpool.tile([128, 1024], BF16)
                    nc.sync.dma_start(xt[:], xT[:, 1024 * i : 1024 * (i + 1)])
                    # fills are deferred 2 chunks so their dependency waits
                    # never stall upcoming xT loads queued behind them
                    if pending_fills and i >= pending_fills[0][0] + \
                            pending_fills[0][1] + 2:
                        emit_fill(*pending_fills.pop(0))
                    z = pz.tile([128, 512], F32)
                    nc.tensor.matmul(z[0:64, :], lhsT=W1_sb[:],
                                     rhs=xt[:, 0:512], start=True, stop=True)
                    nc.tensor.matmul(z[64:128, :], lhsT=W1_sb[:],
                                     rhs=xt[:, 512:1024], start=True, stop=True)
                    th = thpool.tile([128, 512], BF16)
                    nc.scalar.activation(th[:], z[:],
                                         mybir.ActivationFunctionType.Tanh,
                                         bias=b1_sb[:, 0:1])
                    sAB = psd.tile([128, 512], F32, tag="sAB")
                    nc.tensor.matmul(sAB[:], lhsT=W2_sb[:], rhs=th[:],
                                     start=True, stop=True)
                    # copies split to balance scalar vs vector engine time
                    cA0 = nc.scalar.activation(
                        table_sb[0:64, 1024 * i : 1024 * i + 256],
                        sAB[0:64, 0:256],
                        mybir.ActivationFunctionType.Copy)
                    cA1 = nc.vector.tensor_copy(
                        table_sb[0:64, 1024 * i + 256 : 1024 * i + 512],
                        sAB[0:64, 256:512])
                    cB = nc.vector.tensor_copy(
                        table_sb[64:128, 1024 * i + 512 : 1024 * (i + 1)],
                        sAB[64:128, :])
                    table_insts.append(cA0)
                    table_insts.append(cA1)
                    table_insts.append(cB)
                    if (i + 1) % FILLGRP == 0 or i == ACHUNKS - 1:
                        j0 = (i // FILLGRP) * FILLGRP
                        pending_fills.append((j0, i - j0 + 1))
                for f in pending_fills:
                    emit_fill(*f)

            tblv = table_sb[:].rearrange("p (n d) -> p n d", d=2)

            # ---- Phase B: per-batch ap_gather + restripe + mask-reduce ----
            with (
                tc.tile_pool(name="gat", bufs=3) as gpool,
                tc.tile_pool(name="rts", bufs=2) as rpool,
            ):
                cb = 0
                last_rd = [None, None, None]   # per g slot: restripe DMA
                last_red = [None, None]        # per rt slot: reduce inst
                for nb, (k0, B, W) in enumerate(batches):
                    ni = 16 * B * W
                    g = gpool.tile([128, ni * 2], BF16, tag="g")
                    gi = nc.gpsimd.ap_gather(
                        out_ap=g[:].rearrange("p (n d) -> p n d", d=2),
                        in_ap=tblv,
                        idxs_ap=idx_sb[:, cb : cb + B * W],
                        channels=128,
                        num_elems=NPAIR,
                        d=2,
                        num_idxs=ni,
                    )
                    if nb == 0:
                        for bi in table_insts:
                            add_dep_helper(gi.ins, bi.ins, reason="table RAW")
                        add_dep_helper(gi.ins, idx_ld.ins, reason="idx RAW")
                    if last_rd[nb % 3] is not None:
                        add_dep_helper(gi.ins, last_rd[nb % 3].ins,
                                       reason="g WAR")

                    rt = rpool.tile([128, B * 2 * W], BF16, tag="rt")
                    # one strided-partition restripe DMA: src reads the 8
                    # group-leader partitions; stream is (r, b, w) so the
                    # per-(group, r) run of B*2W values is contiguous
                    src = g[0:128:16].rearrange("c (r f) -> c r f", r=16)
                    rd = nc.sync.dma_start(rt[:], src)
                    add_dep_helper(rd.ins, gi.ins, reason="gather RAW")
                    if last_red[nb % 2] is not None:
                        add_dep_helper(rd.ins, last_red[nb % 2].ins,
                                       reason="rt WAR")
                    last_rd[nb % 3] = rd

                    nc.vector.tensor_tensor(
                        out=rt[:], in0=rt[:],
                        in1=msk_sb[:, 2 * cb : 2 * (cb + B * W)],
                        op=mybir.AluOpType.mult,
                    )
                    red = nc.vector.tensor_reduce(
                        out=out_sb[:, k0 : k0 + B],
                        in_=rt[:].rearrange("p (b w) -> p b w", b=B),
                        axis=mybir.AxisListType.X,
                        op=mybir.AluOpType.add,
                    )
                    last_red[nb % 2] = red
                    cb += B * W

                nc.sync.dma_start(outd[:, :], out_sb[:])
    nc.compile()
    return nc


_PROGRAM_CACHE = {}


def _get_program(batches):
    key = tuple(batches)
    if key not in _PROGRAM_CACHE:
        _PROGRAM_CACHE[key] = _build_program(key)
    return _PROGRAM_CACHE[key]


def _preprocess(x, edge_row, edge_col, edge_val):
    """Build per-core idx/mask streams and the row permutation.

    Returns (xT, batches, idx_cores, msk_cores, orders) where orders[k] is
    the degree-descending row order of core k (device column-major rank ->
    local row id) and batches is a tuple of (k0, B, W) descriptors.
    """
    xT = np.zeros((128, NPAD), ml_dtypes.bfloat16)
    xT[:, :N] = x.T.astype(ml_dtypes.bfloat16)

    order_e = np.argsort(edge_row, kind="stable")
    ers = edge_row[order_e]
    ecs = edge_col[order_e].astype(np.int64)
    evs = edge_val[order_e]

    deg = np.bincount(ers, minlength=N)
    starts = np.zeros(N + 1, np.int64)
    np.cumsum(deg, out=starts[1:])
    pos = (np.arange(E, dtype=np.int64) - starts[ers]).astype(np.int64)

    # per-core degree-sorted rank of each row
    Wk = np.zeros((NCORES, NCHUNK), np.int64)
    ranks = []   # per core: local row id -> rank
    orders = []  # per core: rank -> local row id
    for k in range(NCORES):
        dk = np.zeros(RPAD, np.int64)
        dk[:RPC] = deg[k * RPC : (k + 1) * RPC]
        o = np.argsort(-dk, kind="stable")
        r = np.empty(RPAD, np.int64)
        r[o] = np.arange(RPAD)
        orders.append(o)
        ranks.append(r)
        Wk[k] = dk[o].reshape(NCHUNK, 128).max(axis=1)
    Wc = np.maximum(Wk.max(axis=0), 1)   # non-increasing (degree-sorted)

    # greedy batch packing with uniform width per batch
    batches = []
    k = 0
    while k < NCHUNK:
        Wb = int(Wc[k])
        B = 1
        while (k + B < NCHUNK and (B + 1) * Wb <= BATCH_CAP
               and B < BATCH_MAXB):
            B += 1
        if (B * Wb) % 2:
            Wb += 1  # keep every batch's idx column offset even
        batches.append((k, B, Wb))
        k += B
    batches = tuple(batches)

    nbat = len(batches)
    A_k0 = np.array([b[0] for b in batches], np.int64)
    A_B = np.array([b[1] for b in batches], np.int64)
    A_W = np.array([b[2] for b in batches], np.int64)
    A_BW = A_B * A_W
    A_cb = np.zeros(nbat, np.int64)
    np.cumsum(A_BW[:-1], out=A_cb[1:])
    sumc = int(A_BW.sum())
    chunk2batch = np.zeros(NCHUNK, np.int64)
    for nb, (k0, B, W) in enumerate(batches):
        chunk2batch[k0 : k0 + B] = nb

    # per-edge placement
    core = ers // RPC
    rloc = ers - core * RPC
    idx_cores = []
    msk_cores = []
    for k in range(NCORES):
        m = core == k
        rk = ranks[k][rloc[m]]           # rank of dest row
        ck = rk // 128                   # chunk
        P = rk % 128                     # lane/partition
        w = pos[m]                       # slot within row
        cols = ecs[m]
        vals = evs[m]
        nb = chunk2batch[ck]
        b = ck - A_k0[nb]
        # group stream position: (r, b, w) row-major within the batch
        j = (P % 16) * A_BW[nb] + b * A_W[nb] + w
        idx_h = np.zeros((128, sumc), np.int16)
        idx_h[16 * (P // 16) + j % 16, A_cb[nb] + j // 16] = (
            cols >> 1).astype(np.int16)
        msk_h = np.zeros((128, 2 * sumc), ml_dtypes.bfloat16)
        msk_h[P, 2 * A_cb[nb] + 2 * (b * A_W[nb] + w) + (cols & 1)] = (
            vals.astype(ml_dtypes.bfloat16))
        idx_cores.append(idx_h)
        msk_cores.append(msk_h)
    return xT, batches, idx_cores, msk_cores, orders


def kernel(x, edge_row, edge_col, edge_val, W1, b1, W2, b2):
    x = np.asarray(x, np.float32)
    edge_row = np.asarray(edge_row, np.int32)
    edge_col = np.asarray(edge_col, np.int32)
    edge_val = np.asarray(edge_val, np.float32)
    W1 = np.asarray(W1, np.float32)
    b1 = np.asarray(b1, np.float32)
    W2 = np.asarray(W2, np.float32)
    b2 = np.asarray(b2, np.float32)

    xT, batches, idx_cores, msk_cores, orders = _preprocess(
        x, edge_row, edge_col, edge_val
    )
    nc = _get_program(batches)

    W1h = np.ascontiguousarray(W1.astype(ml_dtypes.bfloat16))
    b1c = np.tile(b1, 2).reshape(128, 1)
    W2b = np.zeros((128, 128), ml_dtypes.bfloat16)
    W2b[0:64, 0:64] = W2[:, 0:1].astype(ml_dtypes.bfloat16)
    W2b[64:128, 64:128] = W2[:, 0:1].astype(ml_dtypes.bfloat16)

    in_maps = [
        {
            "xT": xT,
            "W1": W1h,
            "b1c": b1c,
            "W2blk": W2b,
            "idx": idx_cores[k],
            "msk": msk_cores[k],
        }
        for k in range(NCORES)
    ]
    trace = bool(int(os.environ.get("GCN_TRACE", "0")))
    if trace:
        _install_ntff_hook()
    try:
        res = run_bass_kernel_spmd(
            nc,
            in_maps,
            core_ids=list(range(NCORES)),
            trace=trace,
        )
    except Exception:
        # transient device/runtime hiccups (e.g. a wedged core from a prior
        # session) have been observed once; a clean retry recovers
        res = run_bass_kernel_spmd(
            nc,
            in_maps,
            core_ids=list(range(NCORES)),
            trace=trace,
        )
    _LAST_RESULTS["exec_time_ns"] = res.exec_time_ns
    _LAST_RESULTS["trace"] = res.instructions_and_trace
    _LAST_RESULTS["scope_times"] = res.per_core_scope_times

    out = np.empty((N, 1), np.float32)
    b2f = float(b2.reshape(-1)[0])
    for k in range(NCORES):
        o = np.asarray(res.results[k]["out"], np.float32)  # [128, NCHUNK]
        full = np.empty(RPAD, np.float32)
        full[orders[k]] = o.T.reshape(RPAD)  # rank ck*128+P -> row order[rank]
        out[k * RPC : (k + 1) * RPC, 0] = full[:RPC] + b2f
    return out


# revision 11
# speedup vs baseline: 3.4140x; 3.4140x over previous
"""Trainium2 Bass kernel for ClassicalGCN message passing (bucket-gather v3).

Reference computation:
    h   = tanh(x @ W1 + b1)                       # [N, HID]
    agg = segment_sum(edge_val * h[edge_col], edge_row, N)
    out = agg @ W2 + b2                           # [N, 1]

Algebraic rewrite: W2 commutes through the aggregation:
    s      = tanh(x @ W1 + b1) @ W2               # [N] per-node scalar
    out[i] = b2 + sum_{e: row[e]==i} val[e] * s[col[e]]

Phase A (per 1024-node chunk, 3 PE matmuls): two x@W1 halves, fused tanh,
one block-diagonal W2 matmul; half-copies + periodic cross-partition fill
DMAs replicate s across all 128 partitions as a bf16 table.

Phase B (bucket gather): ap_gather's cost is ~28ns per INDEX regardless of
d (bytes fetched per index), while local_scatter is ~3.5ns per element.
So instead of one gather index per edge (25k/Q7-core), gather 128-node
BUCKETS with d=8: the interleaved table tbl8[16g+q, 8b+l] = s[128b+8q+l]
makes one bucket index deliver 16 different bytes to each of a group's 16
partitions. Each edge's s value lands on partition (col%128)//8 at a
host-assigned (slot, lane). A mask multiply forms val*s products, then a
local_scatter chain demuxes each partition's products into per-row bins
(K=2 per row + overflow levels), DVE reduces bins, and one PE matmul with
a group-indicator matrix sums the 16 partitions of each group into
out[8, 784] (group-major row layout).
"""

import os

import numpy as np
import ml_dtypes

import concourse.bass as bass
import concourse.mybir as mybir
import concourse.tile as tile
from concourse import bacc
from concourse.bass_utils import run_bass_kernel_spmd
from concourse.tile_rust import add_dep_helper

N = 50000
E = 1600000
IN_DIM = 128
HID = 64
NCORES = 8

RPC = N // NCORES            # rows per core = 6250
RPAD = 6272                  # rows padded (49*128)
NCHUNK = RPAD // 128
RPG = RPAD // 8              # rows per 16-partition group = 784
NPAD = 50176                 # nodes padded (49*1024)
ACHUNKS = NPAD // 1024
FILLGRP = 7

D = 8                        # values per bucket-channel (bucket = 16*D nodes)
BUCKET = 16 * D              # 128 nodes per bucket
NB = NPAD // BUCKET          # 392 buckets
ROUNDS = 2
KBIN = 2                     # bin slots per row per scatter level
NBIN = RPG * KBIN            # 1568
NOV = 2046 - NBIN            # 478 overflow slots
NLVL_MIN = 3                 # scatter levels per round (grown per data)

F32 = mybir.dt.float32
BF16 = mybir.dt.bfloat16
I16 = mybir.dt.int16

_LAST_RESULTS = {"exec_time_ns": None}


def _install_ntff_hook():
    import sys

    if "antenv.axon_hooks" in sys.modules:
        return
    import contextlib
    import ctypes
    import types

    so_path = "/opt/axon/libaxon_pjrt.so"
    if not os.path.exists(so_path):
        return
    try:
        lib = ctypes.CDLL(so_path)
    except OSError:
        return
    if not hasattr(lib, "axon_start_nrt_profile"):
        return
    lib.axon_start_nrt_profile.argtypes = [
        ctypes.POINTER(ctypes.c_int64),
        ctypes.c_size_t,
    ]
    lib.axon_start_nrt_profile.restype = ctypes.c_int64
    lib.axon_stop_nrt_profile.argtypes = [ctypes.c_char_p]
    lib.axon_stop_nrt_profile.restype = ctypes.c_int64

    @contextlib.contextmanager
    def _hook(output_dir, device_ids):
        import jax

        jax.devices()
        if device_ids:
            ids = (ctypes.c_int64 * len(device_ids))(*device_ids)
            rc = lib.axon_start_nrt_profile(ids, len(device_ids))
        else:
            rc = lib.axon_start_nrt_profile(None, 0)
        if rc != 0:
            raise RuntimeError(f"axon_start_nrt_profile rc={rc}")
        try:
            yield
        finally:
            n = lib.axon_stop_nrt_profile(str(output_dir).encode())
            if n < 0:
                raise RuntimeError(f"axon_stop_nrt_profile rc={n}")
            print(f"profile: {n} file(s) written to {output_dir}")

    mod = types.ModuleType("antenv.axon_hooks")
    mod.get_axon_ntff_profile_hook = lambda: _hook
    mod.set_axon_ntff_profile_hook = lambda h: None
    sys.modules["antenv.axon_hooks"] = mod


def _build_program(T1, nlvl, sim_tbl8=False):
    """T1: gather indices per round; nlvl: scatter levels per round."""
    TT = ROUNDS * T1
    nc = bacc.Bacc("TRN2", target_bir_lowering=False, debug=False)

    xT = nc.dram_tensor("xT", [128, NPAD], BF16, kind="ExternalInput")
    W1 = nc.dram_tensor("W1", [128, HID], BF16, kind="ExternalInput")
    b1c = nc.dram_tensor("b1c", [128, 1], F32, kind="ExternalInput")
    W2blk = nc.dram_tensor("W2blk", [128, 128], BF16, kind="ExternalInput")
    G8 = nc.dram_tensor("G8", [128, 8], BF16, kind="ExternalInput")
    gidx = nc.dram_tensor("gidx", [128, TT // 16], I16, kind="ExternalInput")
    msk = nc.dram_tensor("msk", [128, D * TT], BF16, kind="ExternalInput")
    s0i = nc.dram_tensor("s0i", [128, D * TT], I16, kind="ExternalInput")
    sov = nc.dram_tensor("sov", [128, (nlvl - 1) * ROUNDS * NOV], I16,
                         kind="ExternalInput")
    outd = nc.dram_tensor("out", [8, RPG], F32, kind="ExternalOutput")
    tbl8d = (nc.dram_tensor("tbl8d", [128, D * NB], BF16,
                            kind="ExternalInput") if sim_tbl8 else None)

    with tile.TileContext(nc) as tc:
        with (
            tc.tile_pool(name="const", bufs=1) as cpool,
        ):
            W1_sb = cpool.tile([128, HID], BF16)
            nc.sync.dma_start(W1_sb[:], W1[:, :])
            b1_sb = cpool.tile([128, 1], F32)
            nc.sync.dma_start(b1_sb[:], b1c[:, :])
            W2_sb = cpool.tile([128, 128], BF16)
            nc.sync.dma_start(W2_sb[:], W2blk[:, :])
            G8_sb = cpool.tile([128, 8], BF16)
            nc.sync.dma_start(G8_sb[:], G8[:, :])

            gidx_sb = cpool.tile([128, TT // 16], I16)
            gidx_ld = nc.sync.dma_start(gidx_sb[:], gidx[:, :])
            msk_sb = cpool.tile([128, D * TT], BF16)
            nc.scalar.dma_start(msk_sb[:], msk[:, :])
            s0i_sb = cpool.tile([128, D * TT], I16)
            s0_ld = nc.scalar.dma_start(s0i_sb[:], s0i[:, :])
            sov_sb = cpool.tile([128, (nlvl - 1) * ROUNDS * NOV], I16)
            sov_ld = nc.scalar.dma_start(sov_sb[:], sov[:, :])

            table_sb = cpool.tile([128, NPAD], BF16)
            tbl8_sb = cpool.tile([128, D * NB], BF16)

            # warm up the GPSIMD library IRAM off the critical path
            dtbl = cpool.tile([128, 32], BF16)
            m0 = nc.vector.memset(dtbl[:], 0)
            didx = cpool.tile([128, 1], I16)
            m1 = nc.vector.memset(didx[:], 0)
            dout = cpool.tile([128, 32], BF16)
            dg = nc.gpsimd.ap_gather(
                out_ap=dout[:].rearrange("p (n d) -> p n d", d=2),
                in_ap=dtbl[:].rearrange("p (n d) -> p n d", d=2),
                idxs_ap=didx[:, :],
                channels=128,
                num_elems=16,
                d=2,
                num_idxs=16,
            )
            add_dep_helper(dg.ins, m0.ins, reason="dummy tbl init")
            add_dep_helper(dg.ins, m1.ins, reason="dummy idx init")

            # ---- Phase A: replicated s table -------------------------------
            table_insts = []

            def emit_fill(j0, ng):
                lo = table_sb[0:64, 1024 * j0 : 1024 * (j0 + ng)]
                hi = table_sb[64:128, 1024 * j0 : 1024 * (j0 + ng)]
                lov = lo.rearrange("p (g c) -> p g c", g=ng)
                hiv = hi.rearrange("p (g c) -> p g c", g=ng)
                fA = nc.sync.dma_start(hiv[:, :, 0:512], lov[:, :, 0:512])
                fB = nc.sync.dma_start(lov[:, :, 512:1024],
                                       hiv[:, :, 512:1024])
                table_insts.append(fA)
                table_insts.append(fB)

            pending_fills = []
            with (
                tc.tile_pool(name="xload", bufs=3) as xpool,
                tc.tile_pool(name="thp", bufs=2) as thpool,
                tc.tile_pool(name="pz", bufs=2, space="PSUM") as pz,
                tc.tile_pool(name="psd", bufs=2, space="PSUM") as psd,
            ):
                for i in range(ACHUNKS):
                    xt = xpool.tile([128, 1024], BF16)
                    nc.sync.dma_start(xt[:], xT[:, 1024 * i : 1024 * (i + 1)])
                    if pending_fills and i >= pending_fills[0][0] + \
                            pending_fills[0][1] + 2:
                        emit_fill(*pending_fills.pop(0))
                    z = pz.tile([128, 512], F32)
                    nc.tensor.matmul(z[0:64, :], lhsT=W1_sb[:],
                                     rhs=xt[:, 0:512], start=True, stop=True)
                    nc.tensor.matmul(z[64:128, :], lhsT=W1_sb[:],
                                     rhs=xt[:, 512:1024], start=True, stop=True)
                    th = thpool.tile([128, 512], BF16)
                    nc.scalar.activation(th[:], z[:],
                                         mybir.ActivationFunctionType.Tanh,
                                         bias=b1_sb[:, 0:1])
                    sAB = psd.tile([128, 512], F32, tag="sAB")
                    nc.tensor.matmul(sAB[:], lhsT=W2_sb[:], rhs=th[:],
                                     start=True, stop=True)
                    cA0 = nc.scalar.activation(
                        table_sb[0:64, 1024 * i : 1024 * i + 256],
                        sAB[0:64, 0:256],
                        mybir.ActivationFunctionType.Copy)
                    cA1 = nc.vector.tensor_copy(
                        table_sb[0:64, 1024 * i + 256 : 1024 * i + 512],
                        sAB[0:64, 256:512])
                    cB = nc.vector.tensor_copy(
                        table_sb[64:128, 1024 * i + 512 : 1024 * (i + 1)],
                        sAB[64:128, :])
                    table_insts.extend((cA0, cA1, cB))
                    if (i + 1) % FILLGRP == 0 or i == ACHUNKS - 1:
                        j0 = (i // FILLGRP) * FILLGRP
                        pending_fills.append((j0, i - j0 + 1))
                for f in pending_fills:
                    emit_fill(*f)

            # interleaved bucket table: tbl8[16g+q, 8b+l] = s[128b + 8q + l]
            tbl_insts = []
            SIXT = NPAD // 16
            if sim_tbl8:
                ti = nc.sync.dma_start(tbl8_sb[:], tbl8d[:, :])
                tbl_insts.append(ti)
            else:
                for q in range(16):
                    srcv = table_sb[q:128:16, SIXT * q : SIXT * (q + 1)]
                    dst = tbl8_sb[q:128:16, :]
                    eng = nc.sync if q % 2 == 0 else nc.scalar
                    ti = eng.dma_start(dst, srcv)
                    for bi in table_insts:
                        add_dep_helper(ti.ins, bi.ins, reason="table RAW")
                    tbl_insts.append(ti)

            # ---- Phase B: bucket gather + mask + scatter chain ------------
            tblv = tbl8_sb[:].rearrange("p (n d) -> p n d", d=D)
            contrib = cpool.tile([128, RPG], F32)
            rtmp = cpool.tile([128, RPG], F32)
            cbf = cpool.tile([128, RPG], BF16)

            with (
                tc.tile_pool(name="gat", bufs=1) as gpool,
                tc.tile_pool(name="dst", bufs=2) as dpool,
            ):
                first_red = True
                reds = []
                for r in range(ROUNDS):
                    g = gpool.tile([128, D * T1], BF16, tag="g")
                    gi = nc.gpsimd.ap_gather(
                        out_ap=g[:].rearrange("p (n d) -> p n d", d=D),
                        in_ap=tblv,
                        idxs_ap=gidx_sb[:, r * T1 // 16 : (r + 1) * T1 // 16],
                        channels=128,
                        num_elems=NB,
                        d=D,
                        num_idxs=T1,
                    )
                    if r == 0:
                        for ti in tbl_insts:
                            add_dep_helper(gi.ins, ti.ins, reason="tbl8 RAW")
                        add_dep_helper(gi.ins, gidx_ld.ins, reason="gidx RAW")
                    tt = nc.vector.tensor_tensor(
                        out=g[:], in0=g[:],
                        in1=msk_sb[:, r * D * T1 : (r + 1) * D * T1],
                        op=mybir.AluOpType.mult,
                    )
                    lvl_in = g[:]
                    lvl_idx = [s0i_sb[:, r * D * T1 : (r + 1) * D * T1]] + [
                        sov_sb[:, ((lvi - 1) * ROUNDS + r) * NOV :
                               ((lvi - 1) * ROUNDS + r + 1) * NOV]
                        for lvi in range(1, nlvl)]
                    for lv in range(nlvl):
                        dstt = dpool.tile([128, 2046], BF16, tag="d")
                        sc = nc.gpsimd.local_scatter(
                            out_ap=dstt[:],
                            data_ap=lvl_in,
                            idxs_ap=lvl_idx[lv],
                            channels=128,
                            num_elems=2046,
                            num_idxs=lvl_in.shape[1],
                        )
                        add_dep_helper(sc.ins, tt.ins, reason="prod RAW")
                        add_dep_helper(sc.ins,
                                       (s0_ld if lv == 0 else sov_ld).ins,
                                       reason="scidx RAW")
                        if len(reds) >= 2:
                            add_dep_helper(sc.ins, reds[-2].ins,
                                           reason="dst WAR")
                        red = nc.vector.tensor_reduce(
                            out=(contrib if first_red else rtmp)[:, :],
                            in_=dstt[:, 0:NBIN].rearrange(
                                "p (r k) -> p r k", k=KBIN),
                            axis=mybir.AxisListType.X,
                            op=mybir.AluOpType.add,
                        )
                        add_dep_helper(red.ins, sc.ins, reason="bins RAW")
                        reds.append(red)
                        if not first_red:
                            nc.vector.tensor_tensor(
                                out=contrib[:], in0=contrib[:], in1=rtmp[:],
                                op=mybir.AluOpType.add)
                        first_red = False
                        lvl_in = dstt[:, NBIN:2046]

            with tc.tile_pool(name="po", bufs=1, space="PSUM") as pout:
                nc.vector.tensor_copy(cbf[:], contrib[:])
                psA = pout.tile([8, 512], F32, tag="pA")
                psB = pout.tile([8, RPG - 512], F32, tag="pB")
                nc.tensor.matmul(psA[:], lhsT=G8_sb[:],
                                 rhs=cbf[:, 0:512], start=True, stop=True)
                nc.tensor.matmul(psB[:], lhsT=G8_sb[:],
                                 rhs=cbf[:, 512:RPG], start=True, stop=True)
                osb = cpool.tile([8, RPG], F32)
                nc.scalar.activation(osb[:, 0:512], psA[:],
                                     mybir.ActivationFunctionType.Copy)
                nc.scalar.activation(osb[:, 512:RPG], psB[:],
                                     mybir.ActivationFunctionType.Copy)
                nc.sync.dma_start(outd[:, :], osb[:])
    nc.compile()
    return nc


_PROGRAM_CACHE = {}


def _get_program(T1, nlvl):
    key = (T1, nlvl)
    if key not in _PROGRAM_CACHE:
        _PROGRAM_CACHE[key] = _build_program(T1, nlvl)
    return _PROGRAM_CACHE[key]


def _seqrank(keys):
    """Rank of each element within its equal-key class (order-stable)."""
    order = np.argsort(keys, kind="stable")
    sk = keys[order]
    n = len(sk)
    newgrp = np.ones(n, bool)
    if n > 1:
        newgrp[1:] = sk[1:] != sk[:-1]
    starts = np.flatnonzero(newgrp)
    gid = np.cumsum(newgrp) - 1
    rank_sorted = np.arange(n) - starts[gid]
    rank = np.empty(n, np.int64)
    rank[order] = rank_sorted
    return rank


def _preprocess(x, edge_row, edge_col, edge_val):
    """Build per-core gather/mask/scatter streams.

    Returns (xT, T1, per_core) where per_core[k] is a dict of host arrays.
    """
    xT = np.zeros((128, NPAD), ml_dtypes.bfloat16)
    xT[:, :N] = x.T.astype(ml_dtypes.bfloat16)

    ers = edge_row.astype(np.int64)
    ecs = edge_col.astype(np.int64)
    evs = edge_val

    core = ers // RPC
    rloc_g = ers - core * RPC            # local row id 0..6249

    # First pass per core: bucket slot structure (needs global T1)
    pre = []
    maxT = 0
    for k in range(NCORES):
        m = core == k
        R = rloc_g[m]
        C = ecs[m]
        V = evs[m]
        g = (R % 128) // 16              # group 0..7
        b = (C % (NPAD // 16)) // D      # bucket-slot within sixteenth
        # per (g, node) multiplicity rank -> slot within bucket
        rank = _seqrank(g * NPAD + C)
        Mgb = np.zeros((8, NB), np.int64)
        np.maximum.at(Mgb, (g, b), rank + 1)
        pre.append(dict(R=R, C=C, V=V, g=g, b=b, rank=rank, Mgb=Mgb))
        maxT = max(maxT, int(Mgb.sum(axis=1).max()))

    # uniform per-round stream length, 32-aligned
    T1 = -(-(-(-maxT // 2)) // 32) * 32
    TT = ROUNDS * T1

    per_core = []
    for k in range(NCORES):
        p = pre[k]
        g, b, rank, Mgb = p["g"], p["b"], p["rank"], p["Mgb"]
        R, C, V = p["R"], p["C"], p["V"]
        q = C // (NPAD // 16)            # channel = sixteenth of node space
        l = C % D                        # lane 0..D-1
        # bucket slot bases per group (slots run 0..T_g)
        base = np.zeros((8, NB), np.int64)
        base[:, 1:] = np.cumsum(Mgb[:, :-1], axis=1)
        slot = base[g, b] + rank         # slot in [0, T_g)
        rnd = slot % 2                   # round (interleaved for balance)
        t_in_r = slot // 2               # slot within round

        # gather idx stream: element j of round r -> partition 16g + j%16,
        # col (r*T1 + j)//16;  value = bucket id
        gidx_h = np.zeros((128, TT // 16), np.int16)
        Tg = Mgb.sum(axis=1)
        for gg in range(8):
            bk = np.repeat(np.arange(NB, dtype=np.int16), Mgb[gg])
            sl = np.arange(Tg[gg], dtype=np.int64)
            rr = sl % 2
            jj = sl // 2
            gidx_h[16 * gg + jj % 16, (rr * T1 + jj) // 16] = bk
        # mask: val at [16g+q, r*D*T1 + D*t + l]  (positions are unique)
        part = 16 * g + q
        pos0 = rnd * (D * T1) + D * t_in_r + l
        msk_h = np.zeros((128, D * TT), np.float32)
        msk_h[part, pos0] = V
        msk_h = msk_h.astype(ml_dtypes.bfloat16)

        # scatter chain: cell = (partition, row, round); rank within cell
        rlocal = (R // 128) * 16 + (R % 16)   # row index within group 0..783
        crank = _seqrank((part * RPG + rlocal) * 2 + rnd)
        lv = crank // KBIN
        kk = crank % KBIN
        binpos = (KBIN * rlocal + kk).astype(np.int16)

        nlvl_k = max(NLVL_MIN, int(lv.max()) + 1)
        s_h = [np.full((128, D * TT), -1, np.int16)] + [
            np.full((128, ROUNDS * NOV), -1, np.int16)
            for _ in range(nlvl_k - 1)]
        pos = pos0
        sel = np.ones(len(lv), bool)
        for lvi in range(nlvl_k):
            # edges surviving to this level, at stream positions pos[sel]
            mb = sel & (lv == lvi)
            s_h[lvi][part[mb], pos[mb]] = binpos[mb]
            mo = sel & (lv > lvi)
            if not mo.any():
                break
            ovr = np.full(len(lv), -1, np.int64)
            ovr[mo] = _seqrank(part[mo] * 2 + rnd[mo])
            assert ovr.max() < NOV, f"overflow exceeded: {ovr.max()}"
            s_h[lvi][part[mo], pos[mo]] = (NBIN + ovr[mo]).astype(np.int16)
            # positions of these edges in the next level's input stream
            nxt = np.full(len(lv), -1, np.int64)
            nxt[mo] = rnd[mo] * NOV + ovr[mo]
            pos = nxt
            sel = mo

        per_core.append(dict(gidx=gidx_h, msk=np.asarray(msk_h),
                             s0i=s_h[0], sov=s_h[1:]))
    return xT, T1, per_core


def kernel(x, edge_row, edge_col, edge_val, W1, b1, W2, b2):
    x = np.asarray(x, np.float32)
    edge_row = np.asarray(edge_row, np.int32)
    edge_col = np.asarray(edge_col, np.int32)
    edge_val = np.asarray(edge_val, np.float32)
    W1 = np.asarray(W1, np.float32)
    b1 = np.asarray(b1, np.float32)
    W2 = np.asarray(W2, np.float32)
    b2 = np.asarray(b2, np.float32)

    xT, T1, per_core = _preprocess(x, edge_row, edge_col, edge_val)
    nlvl = max(NLVL_MIN, max(len(pc["sov"]) + 1 for pc in per_core))
    for pc in per_core:
        ovs = pc.pop("sov")
        full = np.full((128, (nlvl - 1) * ROUNDS * NOV), -1, np.int16)
        for i, arr in enumerate(ovs):
            # level lvi=i+1, round r lives at ((lvi-1)*ROUNDS + r)*NOV
            full[:, i * ROUNDS * NOV : (i + 1) * ROUNDS * NOV] = arr
        pc["sov"] = full
    nc = _get_program(T1, nlvl)

    W1h = np.ascontiguousarray(W1.astype(ml_dtypes.bfloat16))
    b1c = np.tile(b1, 2).reshape(128, 1)
    W2b = np.zeros((128, 128), ml_dtypes.bfloat16)
    W2b[0:64, 0:64] = W2[:, 0:1].astype(ml_dtypes.bfloat16)
    W2b[64:128, 64:128] = W2[:, 0:1].astype(ml_dtypes.bfloat16)
    G8 = np.zeros((128, 8), ml_dtypes.bfloat16)
    for gg in range(8):
        G8[16 * gg : 16 * (gg + 1), gg] = 1

    in_maps = [
        dict(xT=xT, W1=W1h, b1c=b1c, W2blk=W2b, G8=G8, **per_core[k])
        for k in range(NCORES)
    ]
    trace = bool(int(os.environ.get("GCN_TRACE", "0")))
    if trace:
        _install_ntff_hook()
    try:
        res = run_bass_kernel_spmd(
            nc, in_maps, core_ids=list(range(NCORES)), trace=trace)
    except Exception:
        res = run_bass_kernel_spmd(
            nc, in_maps, core_ids=list(range(NCORES)), trace=trace)
    _LAST_RESULTS["exec_time_ns"] = res.exec_time_ns
    _LAST_RESULTS["trace"] = res.instructions_and_trace
    _LAST_RESULTS["scope_times"] = res.per_core_scope_times

    out = np.empty((N, 1), np.float32)
    b2f = float(b2.reshape(-1)[0])
    for k in range(NCORES):
        o = np.asarray(res.results[k]["out"], np.float32)   # [8, RPG]
        full = np.empty(RPAD, np.float32)
        # row R -> (g=(R%128)//16, rloc=(R//128)*16 + R%16)
        Rr = np.arange(RPAD)
        full[Rr] = o[(Rr % 128) // 16, (Rr // 128) * 16 + (Rr % 16)]
        out[k * RPC : (k + 1) * RPC, 0] = full[:RPC] + b2f
    return out


# revision 14
# speedup vs baseline: 3.5353x; 1.0355x over previous
"""Trainium2 Bass kernel for ClassicalGCN message passing (bucket-gather v3).

Reference computation:
    h   = tanh(x @ W1 + b1)                       # [N, HID]
    agg = segment_sum(edge_val * h[edge_col], edge_row, N)
    out = agg @ W2 + b2                           # [N, 1]

Algebraic rewrite: W2 commutes through the aggregation:
    s      = tanh(x @ W1 + b1) @ W2               # [N] per-node scalar
    out[i] = b2 + sum_{e: row[e]==i} val[e] * s[col[e]]

Phase A (per 1024-node chunk, 3 PE matmuls): two x@W1 halves, fused tanh,
one block-diagonal W2 matmul; half-copies + periodic cross-partition fill
DMAs replicate s across all 128 partitions as a bf16 table.

Phase B (bucket gather): ap_gather's cost is ~28ns per INDEX regardless of
d (bytes fetched per index), while local_scatter is ~3.5ns per element.
So instead of one gather index per edge (25k/Q7-core), gather 128-node
BUCKETS with d=8: the interleaved table tbl8[16g+q, 8b+l] = s[128b+8q+l]
makes one bucket index deliver 16 different bytes to each of a group's 16
partitions. Each edge's s value lands on partition (col%128)//8 at a
host-assigned (slot, lane). A mask multiply forms val*s products, then a
local_scatter chain demuxes each partition's products into per-row bins
(K=2 per row + overflow levels), DVE reduces bins, and one PE matmul with
a group-indicator matrix sums the 16 partitions of each group into
out[8, 784] (group-major row layout).
"""

import os

import numpy as np
import ml_dtypes

import concourse.bass as bass
import concourse.mybir as mybir
import concourse.tile as tile
from concourse import bacc
from concourse.bass_utils import run_bass_kernel_spmd
from concourse.tile_rust import add_dep_helper

N = 50000
E = 1600000
IN_DIM = 128
HID = 64
NCORES = 8

RPC = N // NCORES            # rows per core = 6250
RPAD = 6272                  # rows padded (49*128)
NCHUNK = RPAD // 128
RPG = RPAD // 8              # rows per 16-partition group = 784
NPAD = 50176                 # nodes padded (49*1024)
ACHUNKS = NPAD // 1024
FILLGRP = 7

D = 8                        # values per bucket-channel (bucket = 16*D nodes)
BUCKET = 16 * D              # 128 nodes per bucket
NB = NPAD // BUCKET          # 392 buckets
ROUNDS = 2
KBIN = 2                     # bin slots per row per scatter level
NBIN = RPG * KBIN            # 1568
NOV = 2046 - NBIN            # 478 overflow slots
NLVL_MIN = 3                 # scatter levels per round (grown per data)

F32 = mybir.dt.float32
BF16 = mybir.dt.bfloat16
I16 = mybir.dt.int16

_LAST_RESULTS = {"exec_time_ns": None}


def _install_ntff_hook():
    import sys

    if "antenv.axon_hooks" in sys.modules:
        return
    import contextlib
    import ctypes
    import types

    so_path = "/opt/axon/libaxon_pjrt.so"
    if not os.path.exists(so_path):
        return
    try:
        lib = ctypes.CDLL(so_path)
    except OSError:
        return
    if not hasattr(lib, "axon_start_nrt_profile"):
        return
    lib.axon_start_nrt_profile.argtypes = [
        ctypes.POINTER(ctypes.c_int64),
        ctypes.c_size_t,
    ]
    lib.axon_start_nrt_profile.restype = ctypes.c_int64
    lib.axon_stop_nrt_profile.argtypes = [ctypes.c_char_p]
    lib.axon_stop_nrt_profile.restype = ctypes.c_int64

    @contextlib.contextmanager
    def _hook(output_dir, device_ids):
        import jax

        jax.devices()
        if device_ids:
            ids = (ctypes.c_int64 * len(device_ids))(*device_ids)
            rc = lib.axon_start_nrt_profile(ids, len(device_ids))
        else:
            rc = lib.axon_start_nrt_profile(None, 0)
        if rc != 0:
            raise RuntimeError(f"axon_start_nrt_profile rc={rc}")
        try:
            yield
        finally:
            n = lib.axon_stop_nrt_profile(str(output_dir).encode())
            if n < 0:
                raise RuntimeError(f"axon_stop_nrt_profile rc={n}")
            print(f"profile: {n} file(s) written to {output_dir}")

    mod = types.ModuleType("antenv.axon_hooks")
    mod.get_axon_ntff_profile_hook = lambda: _hook
    mod.set_axon_ntff_profile_hook = lambda h: None
    sys.modules["antenv.axon_hooks"] = mod


def _build_program(T1, nlvl, sim_tbl8=False):
    """T1: gather indices per round; nlvl: scatter levels per round."""
    TT = ROUNDS * T1
    nc = bacc.Bacc("TRN2", target_bir_lowering=False, debug=False)

    xT = nc.dram_tensor("xT", [128, NPAD], BF16, kind="ExternalInput")
    W1 = nc.dram_tensor("W1", [128, HID], BF16, kind="ExternalInput")
    b1c = nc.dram_tensor("b1c", [128, 1], F32, kind="ExternalInput")
    W2blk = nc.dram_tensor("W2blk", [128, 128], BF16, kind="ExternalInput")
    G8 = nc.dram_tensor("G8", [128, 8], BF16, kind="ExternalInput")
    gidx = nc.dram_tensor("gidx", [128, TT // 16], I16, kind="ExternalInput")
    msk = nc.dram_tensor("msk", [128, D * TT], BF16, kind="ExternalInput")
    s0i = nc.dram_tensor("s0i", [128, D * TT], I16, kind="ExternalInput")
    sov = nc.dram_tensor("sov", [128, (nlvl - 1) * ROUNDS * NOV], I16,
                         kind="ExternalInput")
    outd = nc.dram_tensor("out", [8, RPG], F32, kind="ExternalOutput")
    tbl8d = (nc.dram_tensor("tbl8d", [128, D * NB], BF16,
                            kind="ExternalInput") if sim_tbl8 else None)

    with tile.TileContext(nc) as tc:
        with (
            tc.tile_pool(name="const", bufs=1) as cpool,
        ):
            W1_sb = cpool.tile([128, HID], BF16)
            nc.sync.dma_start(W1_sb[:], W1[:, :])
            b1_sb = cpool.tile([128, 1], F32)
            nc.sync.dma_start(b1_sb[:], b1c[:, :])
            W2_sb = cpool.tile([128, 128], BF16)
            nc.sync.dma_start(W2_sb[:], W2blk[:, :])
            G8_sb = cpool.tile([128, 8], BF16)
            nc.sync.dma_start(G8_sb[:], G8[:, :])

            gidx_sb = cpool.tile([128, TT // 16], I16)
            msk_sb = cpool.tile([128, D * TT], BF16)
            s0i_sb = cpool.tile([128, D * TT], I16)
            sov_sb = cpool.tile([128, (nlvl - 1) * ROUNDS * NOV], I16)
            preload_state = {}

            def emit_preloads():
                preload_state["gidx"] = nc.sync.dma_start(gidx_sb[:],
                                                          gidx[:, :])
                nc.scalar.dma_start(msk_sb[:], msk[:, :])
                preload_state["s0"] = nc.sync.dma_start(s0i_sb[:], s0i[:, :])
                preload_state["sov"] = nc.scalar.dma_start(sov_sb[:],
                                                           sov[:, :])

            table_sb = cpool.tile([128, NPAD], BF16)
            tbl8_sb = cpool.tile([128, D * NB], BF16)

            # warm up the GPSIMD library IRAM off the critical path
            dtbl = cpool.tile([128, 32], BF16)
            m0 = nc.vector.memset(dtbl[:], 0)
            didx = cpool.tile([128, 1], I16)
            m1 = nc.vector.memset(didx[:], 0)
            dout = cpool.tile([128, 32], BF16)
            dg = nc.gpsimd.ap_gather(
                out_ap=dout[:].rearrange("p (n d) -> p n d", d=2),
                in_ap=dtbl[:].rearrange("p (n d) -> p n d", d=2),
                idxs_ap=didx[:, :],
                channels=128,
                num_elems=16,
                d=2,
                num_idxs=16,
            )
            add_dep_helper(dg.ins, m0.ins, reason="dummy tbl init")
            add_dep_helper(dg.ins, m1.ins, reason="dummy idx init")

            # ---- Phase A: replicated s table -------------------------------
            table_insts = []

            def emit_fill(j0, ng):
                lo = table_sb[0:64, 1024 * j0 : 1024 * (j0 + ng)]
                hi = table_sb[64:128, 1024 * j0 : 1024 * (j0 + ng)]
                lov = lo.rearrange("p (g c) -> p g c", g=ng)
                hiv = hi.rearrange("p (g c) -> p g c", g=ng)
                fA = nc.scalar.dma_start(hiv[:, :, 0:512], lov[:, :, 0:512])
                fB = nc.scalar.dma_start(lov[:, :, 512:1024],
                                         hiv[:, :, 512:1024])
                table_insts.append(fA)
                table_insts.append(fB)

            pending_fills = []
            with (
                tc.tile_pool(name="xload", bufs=4) as xpool,
                tc.tile_pool(name="thp", bufs=2) as thpool,
                tc.tile_pool(name="pz", bufs=2, space="PSUM") as pz,
                tc.tile_pool(name="psd", bufs=2, space="PSUM") as psd,
            ):
                for i in range(ACHUNKS):
                    xt = xpool.tile([128, 1024], BF16)
                    nc.sync.dma_start(xt[:], xT[:, 1024 * i : 1024 * (i + 1)])
                    if i == 3:
                        emit_preloads()
                    if pending_fills and i >= pending_fills[0][0] + \
                            pending_fills[0][1] + 2:
                        emit_fill(*pending_fills.pop(0))
                    z = pz.tile([128, 512], F32)
                    nc.tensor.matmul(z[0:64, :], lhsT=W1_sb[:],
                                     rhs=xt[:, 0:512], start=True, stop=True)
                    nc.tensor.matmul(z[64:128, :], lhsT=W1_sb[:],
                                     rhs=xt[:, 512:1024], start=True, stop=True)
                    th = thpool.tile([128, 512], BF16)
                    nc.scalar.activation(th[:], z[:],
                                         mybir.ActivationFunctionType.Tanh,
                                         bias=b1_sb[:, 0:1])
                    sAB = psd.tile([128, 512], F32, tag="sAB")
                    nc.tensor.matmul(sAB[:], lhsT=W2_sb[:], rhs=th[:],
                                     start=True, stop=True)
                    cA0 = nc.scalar.activation(
                        table_sb[0:64, 1024 * i : 1024 * i + 256],
                        sAB[0:64, 0:256],
                        mybir.ActivationFunctionType.Copy)
                    cA1 = nc.vector.tensor_copy(
                        table_sb[0:64, 1024 * i + 256 : 1024 * i + 512],
                        sAB[0:64, 256:512])
                    cB = nc.vector.tensor_copy(
                        table_sb[64:128, 1024 * i + 512 : 1024 * (i + 1)],
                        sAB[64:128, :])
                    table_insts.extend((cA0, cA1, cB))
                    if (i + 1) % FILLGRP == 0 or i == ACHUNKS - 1:
                        j0 = (i // FILLGRP) * FILLGRP
                        pending_fills.append((j0, i - j0 + 1))
                for f in pending_fills:
                    emit_fill(*f)

            # interleaved bucket table: tbl8[16g+q, 8b+l] = s[128b + 8q + l]
            tbl_insts = []
            SIXT = NPAD // 16
            if sim_tbl8:
                ti = nc.sync.dma_start(tbl8_sb[:], tbl8d[:, :])
                tbl_insts.append(ti)
            else:
                for q in range(16):
                    srcv = table_sb[q:128:16, SIXT * q : SIXT * (q + 1)]
                    dst = tbl8_sb[q:128:16, :]
                    eng = nc.sync if q % 2 == 0 else nc.scalar
                    ti = eng.dma_start(dst, srcv)
                    for bi in table_insts:
                        add_dep_helper(ti.ins, bi.ins, reason="table RAW")
                    tbl_insts.append(ti)

            # ---- Phase B: bucket gather + mask + scatter chain ------------
            tblv = tbl8_sb[:].rearrange("p (n d) -> p n d", d=D)
            contrib = cpool.tile([128, RPG], F32)
            rtmp = cpool.tile([128, RPG], F32)
            cbf = cpool.tile([128, RPG], BF16)

            with (
                tc.tile_pool(name="gat", bufs=1) as gpool,
                tc.tile_pool(name="dst", bufs=2) as dpool,
            ):
                first_red = True
                reds = []
                for r in range(ROUNDS):
                    g = gpool.tile([128, D * T1], BF16, tag="g")
                    gi = nc.gpsimd.ap_gather(
                        out_ap=g[:].rearrange("p (n d) -> p n d", d=D),
                        in_ap=tblv,
                        idxs_ap=gidx_sb[:, r * T1 // 16 : (r + 1) * T1 // 16],
                        channels=128,
                        num_elems=NB,
                        d=D,
                        num_idxs=T1,
                    )
                    if r == 0:
                        for ti in tbl_insts:
                            add_dep_helper(gi.ins, ti.ins, reason="tbl8 RAW")
                        add_dep_helper(gi.ins, preload_state["gidx"].ins,
                                       reason="gidx RAW")
                    tt = nc.vector.tensor_tensor(
                        out=g[:], in0=g[:],
                        in1=msk_sb[:, r * D * T1 : (r + 1) * D * T1],
                        op=mybir.AluOpType.mult,
                    )
                    lvl_in = g[:]
                    lvl_idx = [s0i_sb[:, r * D * T1 : (r + 1) * D * T1]] + [
                        sov_sb[:, ((lvi - 1) * ROUNDS + r) * NOV :
                               ((lvi - 1) * ROUNDS + r + 1) * NOV]
                        for lvi in range(1, nlvl)]
                    for lv in range(nlvl):
                        dstt = dpool.tile([128, 2046], BF16, tag="d")
                        sc = nc.gpsimd.local_scatter(
                            out_ap=dstt[:],
                            data_ap=lvl_in,
                            idxs_ap=lvl_idx[lv],
                            channels=128,
                            num_elems=2046,
                            num_idxs=lvl_in.shape[1],
                        )
                        add_dep_helper(sc.ins, tt.ins, reason="prod RAW")
                        add_dep_helper(sc.ins,
                                       preload_state["s0" if lv == 0
                                                     else "sov"].ins,
                                       reason="scidx RAW")
                        if len(reds) >= 2:
                            add_dep_helper(sc.ins, reds[-2].ins,
                                           reason="dst WAR")
                        red = nc.vector.tensor_reduce(
                            out=(contrib if first_red else rtmp)[:, :],
                            in_=dstt[:, 0:NBIN].rearrange(
                                "p (r k) -> p r k", k=KBIN),
                            axis=mybir.AxisListType.X,
                            op=mybir.AluOpType.add,
                        )
                        add_dep_helper(red.ins, sc.ins, reason="bins RAW")
                        reds.append(red)
                        if not first_red:
                            nc.vector.tensor_tensor(
                                out=contrib[:], in0=contrib[:], in1=rtmp[:],
                                op=mybir.AluOpType.add)
                        first_red = False
                        lvl_in = dstt[:, NBIN:2046]

            with tc.tile_pool(name="po", bufs=1, space="PSUM") as pout:
                nc.vector.tensor_copy(cbf[:], contrib[:])
                psA = pout.tile([8, 512], F32, tag="pA")
                psB = pout.tile([8, RPG - 512], F32, tag="pB")
                nc.tensor.matmul(psA[:], lhsT=G8_sb[:],
                                 rhs=cbf[:, 0:512], start=True, stop=True)
                nc.tensor.matmul(psB[:], lhsT=G8_sb[:],
                                 rhs=cbf[:, 512:RPG], start=True, stop=True)
                osb = cpool.tile([8, RPG], F32)
                nc.scalar.activation(osb[:, 0:512], psA[:],
                                     mybir.ActivationFunctionType.Copy)
                nc.scalar.activation(osb[:, 512:RPG], psB[:],
                                     mybir.ActivationFunctionType.Copy)
                nc.sync.dma_start(outd[:, :], osb[:])
    nc.compile()
    return nc


_PROGRAM_CACHE = {}


def _get_program(T1, nlvl):
    key = (T1, nlvl)
    if key not in _PROGRAM_CACHE:
        _PROGRAM_CACHE[key] = _build_program(T1, nlvl)
    return _PROGRAM_CACHE[key]


def _seqrank(keys):
    """Rank of each element within its equal-key class (order-stable)."""
    order = np.argsort(keys, kind="stable")
    sk = keys[order]
    n = len(sk)
    newgrp = np.ones(n, bool)
    if n > 1:
        newgrp[1:] = sk[1:] != sk[:-1]
    starts = np.flatnonzero(newgrp)
    gid = np.cumsum(newgrp) - 1
    rank_sorted = np.arange(n) - starts[gid]
    rank = np.empty(n, np.int64)
    rank[order] = rank_sorted
    return rank


def _preprocess(x, edge_row, edge_col, edge_val):
    """Build per-core gather/mask/scatter streams.

    Returns (xT, T1, per_core) where per_core[k] is a dict of host arrays.
    """
    xT = np.zeros((128, NPAD), ml_dtypes.bfloat16)
    xT[:, :N] = x.T.astype(ml_dtypes.bfloat16)

    ers = edge_row.astype(np.int64)
    ecs = edge_col.astype(np.int64)
    evs = edge_val

    core = ers // RPC
    rloc_g = ers - core * RPC            # local row id 0..6249

    # First pass per core: bucket slot structure (needs global T1)
    pre = []
    maxT = 0
    for k in range(NCORES):
        m = core == k
        R = rloc_g[m]
        C = ecs[m]
        V = evs[m]
        g = (R % 128) // 16              # group 0..7
        b = (C % (NPAD // 16)) // D      # bucket-slot within sixteenth
        # per (g, node) multiplicity rank -> slot within bucket
        rank = _seqrank(g * NPAD + C)
        Mgb = np.zeros((8, NB), np.int64)
        np.maximum.at(Mgb, (g, b), rank + 1)
        pre.append(dict(R=R, C=C, V=V, g=g, b=b, rank=rank, Mgb=Mgb))
        maxT = max(maxT, int(Mgb.sum(axis=1).max()))

    # uniform per-round stream length, 32-aligned
    T1 = -(-(-(-maxT // 2)) // 32) * 32
    TT = ROUNDS * T1

    per_core = []
    for k in range(NCORES):
        p = pre[k]
        g, b, rank, Mgb = p["g"], p["b"], p["rank"], p["Mgb"]
        R, C, V = p["R"], p["C"], p["V"]
        q = C // (NPAD // 16)            # channel = sixteenth of node space
        l = C % D                        # lane 0..D-1
        # bucket slot bases per group (slots run 0..T_g)
        base = np.zeros((8, NB), np.int64)
        base[:, 1:] = np.cumsum(Mgb[:, :-1], axis=1)
        slot = base[g, b] + rank         # slot in [0, T_g)
        rnd = slot % 2                   # round (interleaved for balance)
        t_in_r = slot // 2               # slot within round

        # gather idx stream: element j of round r -> partition 16g + j%16,
        # col (r*T1 + j)//16;  value = bucket id
        gidx_h = np.zeros((128, TT // 16), np.int16)
        Tg = Mgb.sum(axis=1)
        for gg in range(8):
            bk = np.repeat(np.arange(NB, dtype=np.int16), Mgb[gg])
            sl = np.arange(Tg[gg], dtype=np.int64)
            rr = sl % 2
            jj = sl // 2
            gidx_h[16 * gg + jj % 16, (rr * T1 + jj) // 16] = bk
        # mask: val at [16g+q, r*D*T1 + D*t + l]  (positions are unique)
        part = 16 * g + q
        pos0 = rnd * (D * T1) + D * t_in_r + l
        msk_h = np.zeros((128, D * TT), np.float32)
        msk_h[part, pos0] = V
        msk_h = msk_h.astype(ml_dtypes.bfloat16)

        # scatter chain: cell = (partition, row, round); rank within cell
        rlocal = (R // 128) * 16 + (R % 16)   # row index within group 0..783
        crank = _seqrank((part * RPG + rlocal) * 2 + rnd)
        lv = crank // KBIN
        kk = crank % KBIN
        binpos = (KBIN * rlocal + kk).astype(np.int16)

        nlvl_k = max(NLVL_MIN, int(lv.max()) + 1)
        s_h = [np.full((128, D * TT), -1, np.int16)] + [
            np.full((128, ROUNDS * NOV), -1, np.int16)
            for _ in range(nlvl_k - 1)]
        pos = pos0
        sel = np.ones(len(lv), bool)
        for lvi in range(nlvl_k):
            # edges surviving to this level, at stream positions pos[sel]
            mb = sel & (lv == lvi)
            s_h[lvi][part[mb], pos[mb]] = binpos[mb]
            mo = sel & (lv > lvi)
            if not mo.any():
                break
            ovr = np.full(len(lv), -1, np.int64)
            ovr[mo] = _seqrank(part[mo] * 2 + rnd[mo])
            assert ovr.max() < NOV, f"overflow exceeded: {ovr.max()}"
            s_h[lvi][part[mo], pos[mo]] = (NBIN + ovr[mo]).astype(np.int16)
            # positions of these edges in the next level's input stream
            nxt = np.full(len(lv), -1, np.int64)
            nxt[mo] = rnd[mo] * NOV + ovr[mo]
            pos = nxt
            sel = mo

        per_core.append(dict(gidx=gidx_h, msk=np.asarray(msk_h),
                             s0i=s_h[0], sov=s_h[1:]))
    return xT, T1, per_core


def kernel(x, edge_row, edge_col, edge_val, W1, b1, W2, b2):
    x = np.asarray(x, np.float32)
    edge_row = np.asarray(edge_row, np.int32)
    edge_col = np.asarray(edge_col, np.int32)
    edge_val = np.asarray(edge_val, np.float32)
    W1 = np.asarray(W1, np.float32)
    b1 = np.asarray(b1, np.float32)
    W2 = np.asarray(W2, np.float32)
    b2 = np.asarray(b2, np.float32)

    xT, T1, per_core = _preprocess(x, edge_row, edge_col, edge_val)
    nlvl = max(NLVL_MIN, max(len(pc["sov"]) + 1 for pc in per_core))
    for pc in per_core:
        ovs = pc.pop("sov")
        full = np.full((128, (nlvl - 1) * ROUNDS * NOV), -1, np.int16)
        for i, arr in enumerate(ovs):
            # level lvi=i+1, round r lives at ((lvi-1)*ROUNDS + r)*NOV
            full[:, i * ROUNDS * NOV : (i + 1) * ROUNDS * NOV] = arr
        pc["sov"] = full
    nc = _get_program(T1, nlvl)

    W1h = np.ascontiguousarray(W1.astype(ml_dtypes.bfloat16))
    b1c = np.tile(b1, 2).reshape(128, 1)
    W2b = np.zeros((128, 128), ml_dtypes.bfloat16)
    W2b[0:64, 0:64] = W2[:, 0:1].astype(ml_dtypes.bfloat16)
    W2b[64:128, 64:128] = W2[:, 0:1].astype(ml_dtypes.bfloat16)
    G8 = np.zeros((128, 8), ml_dtypes.bfloat16)
    for gg in range(8):
        G8[16 * gg : 16 * (gg + 1), gg] = 1

    in_maps = [
        dict(xT=xT, W1=W1h, b1c=b1c, W2blk=W2b, G8=G8, **per_core[k])
        for k in range(NCORES)
    ]
    trace = bool(int(os.environ.get("GCN_TRACE", "0")))
    if trace:
        _install_ntff_hook()
    try:
        res = run_bass_kernel_spmd(
            nc, in_maps, core_ids=list(range(NCORES)), trace=trace)
    except Exception:
        res = run_bass_kernel_spmd(
            nc, in_maps, core_ids=list(range(NCORES)), trace=trace)
    _LAST_RESULTS["exec_time_ns"] = res.exec_time_ns
    _LAST_RESULTS["trace"] = res.instructions_and_trace
    _LAST_RESULTS["scope_times"] = res.per_core_scope_times

    out = np.empty((N, 1), np.float32)
    b2f = float(b2.reshape(-1)[0])
    for k in range(NCORES):
        o = np.asarray(res.results[k]["out"], np.float32)   # [8, RPG]
        full = np.empty(RPAD, np.float32)
        # row R -> (g=(R%128)//16, rloc=(R//128)*16 + R%16)
        Rr = np.arange(RPAD)
        full[Rr] = o[(Rr % 128) // 16, (Rr // 128) * 16 + (Rr % 16)]
        out[k * RPC : (k + 1) * RPC, 0] = full[:RPC] + b2f
    return out
